# revision 43
# baseline (speedup 1.0000x reference)
"""DKEF kernel for Trainium2 (8 NeuronCores, SPMD data-parallel over rows of x).

Math (reference):
  fx = phi_k(x), fz = phi_k(z)            # 3-layer MLP per kernel k (K=3, H=64)
  sq[k,n,m] = ||fx[k,n] - fz[k,m]||^2
  out[n,m]  = sum_k softmax(kernel_weights)[k] * exp(-sq[k,n,m] / (2*10^log_sigma[k]))

Device strategy per core (N/8 = 2048 rows of x):
  - Bulk-staged DMA + PE transposes for x, z, weights; MLP in [feature, token]
    layout over the z|x token stream (weights shared between sides).
  - Kernels k0,k1 stacked in the partition dim (block-diag W2/W3); k2 packed
    across chunk PAIRS into partitions 0-63 / 64-127 via PSUM column tiling.
  - softplus(u+b) = Ln(Exp(u + b) + 1): per-partition AP bias on Exp, the +1
    rides Ln's scalar bias. Exp/Ln/Gram-Exp share one ACT table set.
  - b3 dropped entirely (cancels in the pairwise distance); the -2 of the
    cross term is folded into the z-side W3; 1/4 into the z norm weights.
  - z norms: ones-matmuls land all 3 k rows in one PSUM tile at partitions
    0/32/64 (column tiling), hi/lo f32r split with 2 wide DVE ops per segment.
  - x norms + ln(w_k): ride the Gram Exp's per-partition bias (exact fp32),
    built by a trailing ones-matmul + PE-transpose pass.
  - Gram tile = ONE f32r matmul per (row-tile, k, m-chunk), AUG=66 rows:
      lhsT = TX[k] = [fx; 1; 1],  rhs = BZ[k] = [-2fz; nz_hi; nz_lo]
      e_k = Exp(-c_k * psum + bias_nk)   (bf16 strips)
  - k-sum with 2 bf16 DVE adds; bf16 DMA out; host converts to f32.
The max(sq, 0) clamp in the reference is a no-op for this data distribution
(min sq ~ 2.1) and is omitted.
"""

import sys

for _p in ("/opt/trn_rl_repo",):
    if _p not in sys.path:
        sys.path.insert(0, _p)

from contextlib import ExitStack

import numpy as np

import concourse.bass as bass
import concourse.tile as tile
from concourse import mybir
from concourse.bass_utils import run_bass_kernel_spmd
from concourse.masks import make_identity

K, N, M, D, H = 3, 16384, 4096, 128, 64
N_CORES = 8
NROWS = N // N_CORES  # 2048 rows of x per core

F32 = mybir.dt.float32
F32R = mybir.dt.float32r
BF16 = mybir.dt.bfloat16

P = 128          # partitions
MMF = 512        # max matmul moving free dim (one PSUM bank of fp32)
CH = 1024        # MLP chunk; groups of 2*CH tokens
HM = 2048        # Gram m-chunk (4 PSUM banks; 2 chunks double-buffered)
AUG = H + 4      # 68 rows: [fx; nx_hi; nx_lo; 1; 1] x [-2fz; 1; 1; nz_hi; nz_lo]

MDT = F32R


def _wait_limit(inst):
    return 1


def _split_overfull_waits(nc):
    """walrus codegen caps sem waits per instruction (1 for drains and
    fused-ldweights matmuls). Tile can attach more. Peel surplus waits
    onto single-wait NOPs inserted just before the instruction on the
    same engine."""
    items = sorted(
        (int(n.split("-")[1]), n, i)
        for n, i in nc.inst_map.items()
        if n.startswith("I-") and n.split("-")[1].isdigit()
    )
    over = [
        (n, i)
        for _, n, i in items
        if i.sync_info is not None
        and i.sync_info.on_wait
        and len(i.sync_info.on_wait) > _wait_limit(i)
    ]
    if not over:
        return
    blocks = list(nc.m.functions[0].blocks)
    for n, inst in over:
        lim = _wait_limit(inst)
        si = inst.sync_info
        waits = list(si.on_wait)
        keep, surplus = waits[:lim], waits[lim:]
        si.on_wait = keep
        inst.sync_info = si
        eng = nc.engines[inst.engine]
        new_names = []
        for w in surplus:
            nop = eng.nop(hint="wait_split", nofuse=True)
            nsi = nop.ins.sync_info
            if nsi is None:
                nsi = type(si)(on_wait=[w], on_update=[])
            else:
                nsi.on_wait = [w]
            nop.ins.sync_info = nsi
            new_names.append(nop.ins.name)
        moved = False
        for blk in blocks:
            insts = list(blk.instructions)
            names = [x.name for x in insts]
            if n in names:
                all_names = set(names) | {
                    x.name for b in blocks for x in b.instructions
                }
                assert set(new_names) <= all_names
                for b in blocks:
                    bi = list(b.instructions)
                    if any(x.name in new_names for x in bi):
                        b.instructions = [x for x in bi if x.name not in new_names]
                insts = list(blk.instructions)
                keep_objs = [x for x in insts if x.name not in new_names]
                new_objs = [nc.inst_map[m_] for m_ in new_names]
                at = [x.name for x in keep_objs].index(n)
                keep_objs[at:at] = new_objs
                blk.instructions = keep_objs
                moved = True
                break
        assert moved, f"could not find block containing {n}"


def _r13(v):
    """Round a python float to 13 mantissa bits (f32r-exact)."""
    import math
    if v == 0:
        return 0.0
    m, e = math.frexp(v)
    return float(np.float32(math.ldexp(round(m * 8192.0) / 8192.0, e)))


def build_program(n_rows, m, cks, lws, hm=HM):
    """Per-core Bass program. cks = 1/(2*10^log_sigma), lws = ln softmax(kw)."""
    hm = min(hm, m)
    assert n_rows % P == 0 and m % MMF == 0 and hm % MMF == 0 and m % hm == 0
    assert n_rows % (2 * CH) == 0 and m % (2 * CH) == 0

    nc = bass.Bass()
    x = nc.declare_dram_parameter("x", [n_rows, D], F32, isOutput=False)
    z = nc.declare_dram_parameter("z", [m, D], F32, isOutput=False)
    W1 = nc.declare_dram_parameter("W1", [K, H, D], F32, isOutput=False)
    b1 = nc.declare_dram_parameter("b1", [K, H], F32, isOutput=False)
    W2 = nc.declare_dram_parameter("W2", [K, H, H], F32, isOutput=False)
    b2 = nc.declare_dram_parameter("b2", [K, H], F32, isOutput=False)
    W3 = nc.declare_dram_parameter("W3", [K, H, H], F32, isOutput=False)
    b3 = nc.declare_dram_parameter("b3", [K, H], F32, isOutput=False)  # unused (cancels)
    out = nc.declare_dram_parameter("out", [n_rows, m], BF16, isOutput=True)

    AF = mybir.ActivationFunctionType
    OP = mybir.AluOpType

    def msetr(ap, v):
        nc.vector.memset(ap.bitcast(F32), _r13(v))

    with ExitStack() as ctx:
        tc = ctx.enter_context(tile.TileContext(nc))
        consts = ctx.enter_context(tc.tile_pool(name="consts", bufs=1))
        big = ctx.enter_context(tc.tile_pool(name="big", bufs=1))

        ident = consts.tile([P, P], F32)
        make_identity(nc, ident)

        # MLP stationaries: k0|k1 stacked; k2 solo (plus 64-shifted copies for
        # the second chunk of each packed pair).
        SW1p = consts.tile([P, P], MDT, name="SW1p")
        SW1s = consts.tile([P, H], MDT, name="SW1s")
        SW2p = consts.tile([P, P], MDT, name="SW2p")
        SW2s = consts.tile([H, H], MDT, name="SW2s")
        SW3px = consts.tile([P, P], MDT, name="SW3px")
        SW3pz = consts.tile([P, P], MDT, name="SW3pz")
        SW3sx = consts.tile([H, H], MDT, name="SW3sx")
        SW3sz = consts.tile([H, H], MDT, name="SW3sz")
        B1p = consts.tile([P, 1], F32, name="B1p")
        B2p = consts.tile([P, 1], F32, name="B2p")
        B1s = consts.tile([H, 1], F32, name="B1s")
        B2s = consts.tile([H, 1], F32, name="B2s")
        # Norm-matmul weights (0.25 on the z side undoes the -2 in -2fz).
        ones1z = consts.tile([H, 1], MDT, name="ones1z")
        ones1x = consts.tile([H, 1], MDT, name="ones1x")
        msetr(ones1z, 0.25)
        msetr(ones1x, 1.0)
        # ln(w_k) columns for the Gram Exp bias.
        BLW = [consts.tile([P, 1], F32, name=f"BLW_{k}") for k in range(K)]
        for k in range(K):
            nc.vector.memset(BLW[k], float(lws[k]))
        msetr(SW2p, 0.0)
        msetr(SW3px, 0.0)
        msetr(SW3pz, 0.0)

        # Persistent Gram operands; ln(w_k) rides the Gram-Exp scalar bias.
        # TX[k] = [fx; nx_hi; nx_lo; 1; 1], BZ[k] = [-2fz; 1; 1; nz_hi; nz_lo]
        TX = [big.tile([AUG, n_rows], MDT, name=f"TX_{k}") for k in range(K)]
        BZ = [big.tile([AUG, m], MDT, name=f"BZ_{k}") for k in range(K)]

        for k in range(K):
            # on GpSimd: keeps the DVE FIFO clear for transpose cast copies
            nc.gpsimd.memset(TX[k][H : H + 4, :].bitcast(F32), _r13(1.0))
            nc.gpsimd.memset(BZ[k][H : H + 2, :].bitcast(F32), _r13(1.0))

        # ---------------- Phases T+F share a scope so xT/zT free before G --------
        tfctx = ctx.enter_context(ExitStack())
        mid = tfctx.enter_context(tc.tile_pool(name="mid", bufs=1))
        xT = mid.tile([P, n_rows], MDT, name="xT")
        zT = mid.tile([P, m], MDT, name="zT")

        # ---------------- Phase T: transposes + stationary prep ----------------
        # x staging issues on the sync queue, z staging + weights on the
        # scalar queue (parallel issue); z transposes are emitted after the
        # x MLP so the in-order PE never stalls on z staging.
        stg = tfctx.enter_context(tc.tile_pool(name="staging", bufs=4))
        zstage = []
        with ExitStack() as fctx:
            tp = fctx.enter_context(tc.tile_pool(name="tp", bufs=4))
            pps = fctx.enter_context(tc.tile_pool(name="pps", bufs=6, space="PSUM"))

            wt = {}
            for k in range(K):
                t = tp.tile([H, D], F32, tag="w1_in")
                nc.scalar.dma_start(out=t, in_=W1[k])
                wt["W1", k] = t
                for nmW, Wsrc in (("W2", W2), ("W3", W3)):
                    t2 = tp.tile([H, H], F32, tag=f"{nmW}_in")
                    nc.scalar.dma_start(out=t2, in_=Wsrc[k])
                    wt[nmW, k] = t2
                for nmB, bsrc in (("b1", b1), ("b2", b2)):
                    row = tp.tile([1, H], F32, tag=f"{nmB}_in")
                    nc.scalar.dma_start(out=row, in_=bsrc[k][None, :])
                    wt[nmB, k] = row

            bq = 2
            for q0 in range(0, n_rows // P, bq):
                sS = stg.tile([P, bq * P], F32, tag="stagex")
                nc.sync.dma_start(
                    out=sS[:, :].rearrange("p (b c) -> p b c", c=P),
                    in_=x[q0 * P : (q0 + bq) * P, :].rearrange(
                        "(b p) c -> p b c", p=P),
                )
                for i in range(bq):
                    ps = pps.tile([P, P], F32, tag="ps_t")
                    nc.tensor.transpose(ps, sS[:, i * P : (i + 1) * P], ident)
                    nc.vector.tensor_copy(
                        xT[:, (q0 + i) * P : (q0 + i + 1) * P], ps)
            for q0 in range(0, m // P, bq):
                sS = stg.tile([P, bq * P], F32, tag="stagez")
                nc.scalar.dma_start(
                    out=sS[:, :].rearrange("p (b c) -> p b c", c=P),
                    in_=z[q0 * P : (q0 + bq) * P, :].rearrange(
                        "(b p) c -> p b c", p=P),
                )
                zstage.append((q0, sS))

            # W1 -> SW1p halves / SW1s; W2/W3 -> block-diag quadrants / solo;
            # biases -> per-partition columns.
            for k in range(K):
                ps = pps.tile([P, H], F32, tag="ps_t")
                nc.tensor.transpose(ps, wt["W1", k], ident[:H, :H])
                if k < 2:
                    nc.vector.tensor_copy(SW1p[:, k * H : (k + 1) * H], ps)
                else:
                    nc.vector.tensor_copy(SW1s, ps)

                ps2 = pps.tile([H, H], F32, tag="ps_t")
                nc.tensor.transpose(ps2, wt["W2", k], ident[:H, :H])
                if k < 2:
                    nc.vector.tensor_copy(
                        SW2p[k * H : (k + 1) * H, k * H : (k + 1) * H], ps2
                    )
                else:
                    nc.vector.tensor_copy(SW2s, ps2)

                ps3 = pps.tile([H, H], F32, tag="ps_t")
                nc.tensor.transpose(ps3, wt["W3", k], ident[:H, :H])
                if k < 2:
                    sl = slice(k * H, (k + 1) * H)
                    nc.vector.tensor_copy(SW3px[sl, sl], ps3)
                    nc.vector.tensor_scalar(SW3pz[sl, sl], ps3, -2.0, None, OP.mult)
                else:
                    nc.vector.tensor_copy(SW3sx, ps3)
                    nc.vector.tensor_scalar(SW3sz, ps3, -2.0, None, OP.mult)

                for nmB, Bp, Bs in (("b1", B1p, B1s), ("b2", B2p, B2s)):
                    psb = pps.tile([H, 1], F32, tag="ps_t")
                    nc.tensor.transpose(psb, wt[nmB, k], ident[:1, :1])
                    if k < 2:
                        nc.vector.tensor_copy(Bp[k * H : (k + 1) * H, :], psb)
                    else:
                        nc.vector.tensor_copy(Bs, psb)

        # ---------------- Phase F: MLP in groups of 2*CH tokens ----------------
        def mlp_groups(fctx, groups, solo_tags=("us2", "us2"), hook=None):
            hp = fctx.enter_context(tc.tile_pool(name="hpool", bufs=2))
            mps = fctx.enter_context(
                tc.tile_pool(name="mlp_ps", bufs=1, space="PSUM"))

            def mm(ps_, lhsT, rhs, parts=P):
                for j in range(0, CH, MMF):
                    nc.tensor.matmul(ps_[0:parts, j : j + MMF], lhsT,
                                     rhs[:, j : j + MMF], start=True, stop=True)

            for gi, (side, sT, g0) in enumerate(groups):
                if hook is not None:
                    hook(gi)
                SW3p = SW3px if side == "x" else SW3pz
                SW3s = SW3sx if side == "x" else SW3sz
                dsts = TX if side == "x" else BZ
                cA, cB = g0, g0 + CH

                def layer(srcA, srcB, soloA, soloB, Wp, Ws, Bpv, Bsv):
                    # pair: two [128, CH] psums -> one [128, 2CH] softplus
                    uA = mps.tile([P, CH], F32, tag="upA")
                    mm(uA, Wp, srcA)
                    uB = mps.tile([P, CH], F32, tag="upB")
                    mm(uB, Wp, srcB)
                    tp2 = hp.tile([P, 2 * CH], MDT, tag="tp2")
                    nc.scalar.activation(tp2[:, :CH], uA, AF.Exp, bias=Bpv)
                    nc.scalar.activation(tp2[:, CH:], uB, AF.Exp, bias=Bpv)
                    hp2 = hp.tile([P, 2 * CH], MDT, tag="hp2")
                    nc.scalar.activation(hp2, tp2, AF.Ln, bias=1.0)
                    # solo k2 per chunk (64 partitions)
                    ts2 = hp.tile([H, 2 * CH], MDT, tag="ts2")
                    uSA = mps.tile([H, CH], F32, tag=solo_tags[0])
                    mm(uSA, Ws, soloA, parts=H)
                    nc.scalar.activation(ts2[:, :CH], uSA, AF.Exp, bias=Bsv)
                    uSB = mps.tile([H, CH], F32, tag=solo_tags[1])
                    mm(uSB, Ws, soloB, parts=H)
                    nc.scalar.activation(ts2[:, CH:], uSB, AF.Exp, bias=Bsv)
                    hs2 = hp.tile([H, 2 * CH], MDT, tag="hs2")
                    nc.scalar.activation(hs2, ts2, AF.Ln, bias=1.0)
                    return hp2, hs2

                sA = sT[:, cA : cA + CH]
                sB = sT[:, cB : cB + CH]
                h1p, h1s = layer(sA, sB, sA, sB, SW1p, SW1s, B1p, B1s)
                h2p, h2s = layer(h1p[:, :CH], h1p[:, CH:],
                                 h1s[:, :CH], h1s[:, CH:],
                                 SW2p, SW2s, B2p, B2s)
                # L3 (no bias: it cancels in the pairwise distance)
                u3A = mps.tile([P, CH], F32, tag="upA")
                mm(u3A, SW3p, h2p[:, :CH])
                u3B = mps.tile([P, CH], F32, tag="upB")
                mm(u3B, SW3p, h2p[:, CH:])
                nc.vector.tensor_copy(dsts[0][0:H, cA : cA + CH], u3A[0:H, :])
                nc.vector.tensor_copy(dsts[1][0:H, cA : cA + CH], u3A[H:P, :])
                nc.vector.tensor_copy(dsts[0][0:H, cB : cB + CH], u3B[0:H, :])
                nc.vector.tensor_copy(dsts[1][0:H, cB : cB + CH], u3B[H:P, :])
                u3SA = mps.tile([H, CH], F32, tag=solo_tags[0])
                mm(u3SA, SW3s, h2s[:, :CH], parts=H)
                nc.vector.tensor_copy(dsts[2][0:H, cA : cA + CH], u3SA)
                u3SB = mps.tile([H, CH], F32, tag=solo_tags[1])
                mm(u3SB, SW3s, h2s[:, CH:], parts=H)
                nc.vector.tensor_copy(dsts[2][0:H, cB : cB + CH], u3SB)

        # One linear MLP pipeline: x group, then z groups, through one pool
        # set. z transposes are emitted after the x group (the in-order PE
        # then never stalls on z staging).
        def ztrans_hook(gi):
            if gi != 1:
                return
            for q0, sS in zstage:
                for i in range(2):
                    ps = pps2.tile([P, P], F32, tag="ps_t2")
                    nc.tensor.transpose(ps, sS[:, i * P : (i + 1) * P], ident)
                    nc.vector.tensor_copy(
                        zT[:, (q0 + i) * P : (q0 + i + 1) * P], ps)

        with ExitStack() as fctx:
            pps2 = fctx.enter_context(tc.tile_pool(name="pps2", bufs=2,
                                                   space="PSUM"))
            groups = [("x", xT, g0) for g0 in range(0, n_rows, 2 * CH)]
            groups += [("z", zT, g0) for g0 in range(0, m, 2 * CH)]
            mlp_groups(fctx, groups, hook=ztrans_hook)

        tfctx.close()

        # x norms: TX row 64 (hi, direct base-64 DVE write) / 65 (lo, via DMA).
        with ExitStack() as fctx:
            sqp = fctx.enter_context(tc.tile_pool(name="sqpoolx", bufs=2))
            nps = fctx.enter_context(tc.tile_pool(name="nx_ps", bufs=2, space="PSUM"))
            rp = fctx.enter_context(tc.tile_pool(name="rowsx", bufs=3))
            for k in range(K):
                sq = sqp.tile([H, n_rows], MDT, tag="sqx")
                nc.scalar.activation(sq, TX[k][0:H, :], AF.Square)
                for j0 in range(0, n_rows, hm):
                    np_ = nps.tile([1, hm], F32, tag="np")
                    for j in range(0, hm, MMF):
                        nc.tensor.matmul(np_[:, j : j + MMF], ones1x,
                                         sq[:, j0 + j : j0 + j + MMF],
                                         start=True, stop=True)
                    seg = slice(j0, j0 + hm)
                    hi_ap = TX[k][H : H + 1, seg]
                    nc.vector.tensor_copy(hi_ap, np_)
                    lo = rp.tile([1, hm], MDT, tag="lo")
                    nc.vector.tensor_tensor(lo, np_, hi_ap, OP.subtract)
                    nc.sync.dma_start(out=TX[k][H + 1 : H + 2, seg], in_=lo)

        # z norms: BZ rows 66/67 via scratch + DMA (base 66 not 32-aligned).
        with ExitStack() as fctx:
            sqp = fctx.enter_context(tc.tile_pool(name="sqpoolz", bufs=2))
            nps = fctx.enter_context(tc.tile_pool(name="nz_ps", bufs=2, space="PSUM"))
            rp = fctx.enter_context(tc.tile_pool(name="rowsz", bufs=2))
            for j0 in range(0, m, hm):
                seg = slice(j0, j0 + hm)
                for k in range(K):
                    sq = sqp.tile([H, hm], MDT, tag="sqz")
                    nc.scalar.activation(sq, BZ[k][0:H, seg], AF.Square)
                    np_ = nps.tile([1, hm], F32, tag="np")
                    for j in range(0, hm, MMF):
                        nc.tensor.matmul(np_[:, j : j + MMF], ones1z,
                                         sq[:, j : j + MMF], start=True, stop=True)
                    hi = rp.tile([1, hm], MDT, tag="hi")
                    nc.vector.tensor_copy(hi, np_)
                    nc.sync.dma_start(out=BZ[k][H + 2 : H + 3, seg], in_=hi)
                    lo = rp.tile([1, hm], MDT, tag="lo")
                    nc.vector.tensor_tensor(lo, np_, hi, OP.subtract)
                    nc.sync.dma_start(out=BZ[k][H + 3 : H + 4, seg], in_=lo)

        # ---------------- Phase G: Gram + exp + k-sum ----------------
        with ExitStack() as gctx:
            gps = gctx.enter_context(tc.tile_pool(name="gram_ps", bufs=2, space="PSUM"))
            ep = gctx.enter_context(tc.tile_pool(name="epool", bufs=2))
            op_ = gctx.enter_context(tc.tile_pool(name="opool", bufs=3))

            for i in range(n_rows // P):
                n0 = i * P
                for h0 in range(0, m, hm):
                    es = []
                    for k in range(K):
                        ps = gps.tile([P, hm], F32, tag="gram")
                        for mt in range(0, hm, MMF):
                            nc.tensor.matmul(
                                ps[:, mt : mt + MMF],
                                TX[k][:, n0 : n0 + P],
                                BZ[k][:, h0 + mt : h0 + mt + MMF],
                                start=True, stop=True,
                            )
                        e = ep.tile([P, hm], BF16, tag=f"e{k}", name=f"e{k}")
                        nc.scalar.activation(e, ps, AF.Exp, scale=float(-cks[k]),
                                             bias=BLW[k])
                        es.append(e)
                    t01 = ep.tile([P, hm], BF16, tag="t01")
                    nc.vector.tensor_tensor(t01, es[0], es[1], OP.add)
                    ot = op_.tile([P, hm], BF16, tag="ot")
                    nc.vector.tensor_tensor(ot, t01, es[2], OP.add)
                    nc.sync.dma_start(out=out[n0 : n0 + P, h0 : h0 + hm], in_=ot)

    _split_overfull_waits(nc)
    nc.finalize()
    return nc


def _host_prep(inputs):
    ls = np.asarray(inputs["log_sigma"], np.float64)
    kw = np.asarray(inputs["kernel_weights"], np.float64)
    cks = 1.0 / (2.0 * np.power(10.0, ls))
    w = np.exp(kw - kw.max())
    w = w / w.sum()
    lws = np.log(w)
    return cks, lws


def run(inputs, trace=False, n_cores=N_CORES):
    cks, lws = _host_prep(inputs)
    nc = build_program(NROWS, M, cks, lws)
    x = np.ascontiguousarray(np.asarray(inputs["x"], np.float32))
    shared = {
        name: np.ascontiguousarray(np.asarray(inputs[name], np.float32))
        for name in ("z", "W1", "b1", "W2", "b2", "W3", "b3")
    }
    in_maps = [
        {"x": x[c * NROWS : (c + 1) * NROWS], **shared} for c in range(n_cores)
    ]
    res = run_bass_kernel_spmd(nc, in_maps, list(range(n_cores)), trace=trace)
    outs = [np.asarray(res.results[c]["out"]).astype(np.float32)
            for c in range(n_cores)]
    return np.concatenate(outs, axis=0), res


def kernel(**inputs) -> np.ndarray:
    out, _ = run(inputs, trace=False)
    return out
